# revision 8
# baseline (speedup 1.0000x reference)
"""Trainium2 Bass kernel for a 4-layer GRU (T=2048, B=64, IN=512, H=3).

Strategy (data-parallel over batch: 8 cores x 8 batch each):
  Phase 1: gx0 = w_ih0 @ x^T for all tokens via PE matmuls (memory-bound).
  Phase 2: wavefront scan over the 4 layers: two stationary matmuls per
    step produce every gate preactivation (r/nx/nh and z/1-z, for all
    layers at once, biases folded in via a ones-row), followed by
    sigmoid/tanh on ACT and 5 small vector ops on DVE. The h history
    lives column-per-step in one big SBUF tile R (append-only), so each
    step's output needs zero copies; layer 3's rows of R double as the
    kernel output. Compute APs only touch partitions at quad offsets
    (0/32/64/96) per the TRN2 access rule.

R tile row map (48 partitions):  0:12 h (layer l at 3l:3l+3)
  32:41 gx0 [r,z,n]   41 ones (bias row)   rest zero.
"""
import numpy as np
from contextlib import ExitStack

import concourse.bass as bass
import concourse.bacc as bacc
import concourse.mybir as mybir
import concourse.tile as tile
from concourse.bass_utils import run_bass_kernel_spmd

T, B, IN, H, L = 2048, 64, 512, 3, 4
NC_CORES = 8
BS = B // NC_CORES          # 8 batch per core
NTOK = T * BS               # 16384 tokens per core
S = T + L - 1               # 2051 wavefront steps
COLS = 8 * (S + 1)          # 16416 columns in R
F32 = mybir.dt.float32
AF = mybir.ActivationFunctionType

_BUILD_CACHE = {}


def _build_bass():
    if "nc" in _BUILD_CACHE:
        return _BUILD_CACHE["nc"]
    nc = bacc.Bacc(None, target_bir_lowering=False)
    xT_d = nc.dram_tensor("xT", [IN, NTOK], F32, kind="ExternalInput")
    w0T_d = nc.dram_tensor("w0T", [128, 36], F32, kind="ExternalInput")
    lhsA_d = nc.dram_tensor("lhsA", [48, 76], F32, kind="ExternalInput")
    lhsB_d = nc.dram_tensor("lhsB", [48, 44], F32, kind="ExternalInput")
    h0_d = nc.dram_tensor("h0", [12, BS], F32, kind="ExternalInput")
    ones_d = nc.dram_tensor("ones", [1, COLS], F32, kind="ExternalInput")
    out_d = nc.dram_tensor("out", [3, NTOK], F32, kind="ExternalOutput")
    hfin_d = nc.dram_tensor("hfin", [12, 32], F32, kind="ExternalOutput")

    with tile.TileContext(nc) as tc, ExitStack() as ctx:
        singles = ctx.enter_context(tc.tile_pool(name="singles", bufs=1))
        xpool = ctx.enter_context(tc.tile_pool(name="xp", bufs=2))
        p1psum = ctx.enter_context(tc.tile_pool(name="p1ps", bufs=2, space="PSUM"))
        psA = ctx.enter_context(tc.tile_pool(name="psA", bufs=2, space="PSUM"))
        psB = ctx.enter_context(tc.tile_pool(name="psB", bufs=2, space="PSUM"))
        psN = ctx.enter_context(tc.tile_pool(name="psN", bufs=2, space="PSUM"))
        work = ctx.enter_context(tc.tile_pool(name="wk", bufs=3))

        R = singles.tile([48, COLS], F32)
        lhsA = singles.tile([48, 76], F32)
        lhsB = singles.tile([48, 44], F32)
        w0 = singles.tile([128, 36], F32)
        nc.vector.memset(R[:, :], 0.0)
        nc.sync.dma_start(out=lhsA[:], in_=lhsA_d[:])
        nc.sync.dma_start(out=lhsB[:], in_=lhsB_d[:])
        nc.sync.dma_start(out=w0[:], in_=w0T_d[:])
        nc.sync.dma_start(out=R[41:42, :], in_=ones_d[:])
        # initial hidden state, staggered for the wavefront warm-up
        for c in range(4):
            nc.sync.dma_start(out=R[3 * c:12, 8 * c:8 * c + 8], in_=h0_d[3 * c:12, :])

        # ---- Phase 1: gx0 into R[32:41, 0:NTOK] ----
        XBLK = 2048
        for blk in range(NTOK // XBLK):
            tok0 = blk * XBLK
            xts = []
            for c in range(4):
                xt = xpool.tile([128, XBLK], F32, tag=f"x{c}")
                nc.sync.dma_start(
                    out=xt[:], in_=xT_d[128 * c:128 * c + 128, tok0:tok0 + XBLK]
                )
                xts.append(xt)
            for sub in range(XBLK // 512):
                ps1 = p1psum.tile([9, 512], F32)
                for c in range(4):
                    nc.tensor.matmul(
                        ps1[:],
                        w0[:, 9 * c:9 * c + 9],
                        xts[c][:, 512 * sub:512 * sub + 512],
                        start=(c == 0),
                        stop=(c == 3),
                    )
                nc.scalar.copy(
                    R[32:41, tok0 + 512 * sub:tok0 + 512 * sub + 512], ps1[:]
                )

        # ---- Phase 2: wavefront scan ----
        for s in range(S):
            c0 = 8 * s
            pa = psA.tile([76, 8], F32)
            pb = psB.tile([44, 8], F32)
            nc.tensor.matmul(pa[:], lhsA[:], R[0:48, c0:c0 + 8], start=True, stop=True)
            nc.tensor.matmul(pb[:], lhsB[:], R[0:48, c0:c0 + 8], start=True, stop=True)
            rt = work.tile([12, 8], F32, tag="rt")
            nc.scalar.activation(rt[:], pa[0:12, :], AF.Sigmoid)
            zw = work.tile([44, 8], F32, tag="zw")
            nc.scalar.activation(zw[:], pb[0:44, :], AF.Sigmoid)
            tt = work.tile([12, 8], F32, tag="tt")
            nc.vector.tensor_mul(tt[:], rt[:], pa[64:76, :])
            un = work.tile([12, 8], F32, tag="un")
            nc.vector.tensor_mul(un[:], zw[0:12, :], R[0:12, c0:c0 + 8])
            sc = work.tile([12, 8], F32, tag="sc")
            nc.vector.tensor_add(sc[:], tt[:], pa[32:44, :])
            n = psN.tile([12, 8], F32)
            nc.scalar.activation(n[:], sc[:], AF.Tanh)
            a = work.tile([12, 8], F32, tag="a")
            nc.vector.tensor_mul(a[:], zw[32:44, :], n[:])
            rows = min(12, 3 * (s + 1))
            nc.vector.tensor_add(
                R[0:rows, c0 + 8:c0 + 16], a[0:rows, :], un[0:rows, :]
            )

        nc.sync.dma_start(out=out_d[:], in_=R[9:12, 32:32 + NTOK])
        nc.sync.dma_start(out=hfin_d[:], in_=R[0:12, NTOK:NTOK + 32])

    nc.finalize()
    _BUILD_CACHE["nc"] = nc
    return nc


def build_lhs_ab(w_hh0, b_ih0, b_hh0, w_ih_rest, w_hh_rest, b_ih_rest, b_hh_rest):
    """Pack the scan's two stationary matrices.

    lhsA (48,76): cols 0:12 r, 32:44 nx, 64:76 nh.
    lhsB (48,44): cols 0:12 z, 32:44 w = 1-z (negated preactivation).
    Contract rows: 0:12 h, 32:41 gx0 [r,z,n], 41 ones.
    """
    A = np.zeros((48, 76), np.float64)
    Bm = np.zeros((48, 44), np.float64)
    for l in range(L):
        whh = w_hh0 if l == 0 else w_hh_rest[l - 1]
        bih = b_ih0 if l == 0 else b_ih_rest[l - 1]
        bhh = b_hh0 if l == 0 else b_hh_rest[l - 1]
        wih = None if l == 0 else w_ih_rest[l - 1]
        for g in range(3):
            c = 3 * l + g
            for j in range(3):
                A[3 * l + j, c] += whh[g, j]            # r
                A[3 * l + j, 64 + c] += whh[6 + g, j]   # nh
                Bm[3 * l + j, c] += whh[3 + g, j]       # z
                Bm[3 * l + j, 32 + c] -= whh[3 + g, j]  # w
                if l > 0:
                    A[3 * (l - 1) + j, c] += wih[g, j]
                    A[3 * (l - 1) + j, 32 + c] += wih[6 + g, j]
                    Bm[3 * (l - 1) + j, c] += wih[3 + g, j]
                    Bm[3 * (l - 1) + j, 32 + c] -= wih[3 + g, j]
            if l == 0:
                A[32 + g, c] = 1.0       # gx0 r -> r0
                A[38 + g, 32 + c] = 1.0  # gx0 n -> nx0
                Bm[35 + g, c] = 1.0      # gx0 z -> z0
                Bm[35 + g, 32 + c] = -1.0
            A[41, c] = bih[g] + bhh[g]
            A[41, 32 + c] = bih[6 + g]
            A[41, 64 + c] = bhh[6 + g]
            Bm[41, c] = bih[3 + g] + bhh[3 + g]
            Bm[41, 32 + c] = -(bih[3 + g] + bhh[3 + g])
    return (np.ascontiguousarray(A.astype(np.float32)),
            np.ascontiguousarray(Bm.astype(np.float32)))


def kernel(x, hxs, w_ih0, w_hh0, b_ih0, b_hh0, w_ih_rest, w_hh_rest,
           b_ih_rest, b_hh_rest):
    x = np.asarray(x, np.float32)
    hxs = np.asarray(hxs, np.float32)
    lhsA, lhsB = build_lhs_ab(
        np.asarray(w_hh0, np.float64), np.asarray(b_ih0, np.float64),
        np.asarray(b_hh0, np.float64), np.asarray(w_ih_rest, np.float64),
        np.asarray(w_hh_rest, np.float64), np.asarray(b_ih_rest, np.float64),
        np.asarray(b_hh_rest, np.float64),
    )
    w0T = np.ascontiguousarray(
        np.asarray(w_ih0, np.float32).T.reshape(4, 128, 9)
        .transpose(1, 0, 2).reshape(128, 36)
    )
    ones = np.ones((1, COLS), np.float32)
    in_maps = []
    for k in range(NC_CORES):
        sl = slice(BS * k, BS * k + BS)
        xT = np.ascontiguousarray(
            x[:, sl, :].transpose(2, 0, 1).reshape(IN, NTOK)
        )
        h0 = np.ascontiguousarray(
            hxs[:, sl, :].transpose(0, 2, 1).reshape(12, BS)
        )
        in_maps.append({"xT": xT, "w0T": w0T, "lhsA": lhsA, "lhsB": lhsB,
                        "h0": h0, "ones": ones})

    global _LAST_IN_MAPS
    _LAST_IN_MAPS = in_maps
    nc = _build_bass()
    results = run_bass_kernel_spmd(nc, in_maps, list(range(NC_CORES))).results

    outs, hs = [], []
    for k in range(NC_CORES):
        o = results[k]["out"].reshape(3, T, BS).transpose(1, 2, 0)
        outs.append(o)
        hf = results[k]["hfin"]
        hs.append(np.stack(
            [hf[3 * l:3 * l + 3, 8 * l:8 * l + 8].T for l in range(L)]
        ))
    out_full = np.ascontiguousarray(np.concatenate(outs, axis=1), dtype=np.float32)
    h_full = np.ascontiguousarray(np.concatenate(hs, axis=1), dtype=np.float32)
    return out_full, h_full


# revision 19
# speedup vs baseline: 30.6979x; 30.6979x over previous
"""Trainium2 Bass kernel for a 4-layer GRU (T=2048, B=64, IN=512, H=3).

Strategy (data-parallel over batch: 8 cores x 8 batch each):
  Phase 1: gx0 = w_ih0 @ x^T for all tokens via PE matmuls (memory-bound).
  Phase 2: wavefront scan over the 4 layers: two stationary matmuls per
    step produce every gate preactivation (r/nx/nh and z/1-z, for all
    layers at once, biases folded in via a ones-row), followed by
    sigmoid/tanh on ACT and 5 small vector ops on DVE. The h history
    lives column-per-step in one big SBUF tile R (append-only), so each
    step's output needs zero copies; layer 3's rows of R double as the
    kernel output. Compute APs only touch partitions at quad offsets
    (0/32/64/96) per the TRN2 access rule.

R tile row map (48 partitions):  0:12 h (layer l at 3l:3l+3)
  32:41 gx0 [r,z,n]   41 ones (bias row)   rest zero.
"""
import numpy as np
from contextlib import ExitStack

import concourse.bass as bass
import concourse.bacc as bacc
import concourse.mybir as mybir
import concourse.tile as tile
from concourse.bass_utils import run_bass_kernel_spmd

T, B, IN, H, L = 2048, 64, 512, 3, 4
NC_CORES = 8
BS = B // NC_CORES          # 8 batch per core
NTOK = T * BS               # 16384 tokens per core
S = T + L - 1               # 2051 wavefront steps
COLS = 8 * (S + 1)          # 16416 columns in R
F32 = mybir.dt.float32
AF = mybir.ActivationFunctionType

_BUILD_CACHE = {}


def _build_bass(scan_steps=S):
    key = ("nc", scan_steps)
    if key in _BUILD_CACHE:
        return _BUILD_CACHE[key]
    nc = bacc.Bacc(None, target_bir_lowering=False)
    xT_d = nc.dram_tensor("xT", [IN, NTOK], F32, kind="ExternalInput")
    w0T_d = nc.dram_tensor("w0T", [128, 36], F32, kind="ExternalInput")
    lhsA_d = nc.dram_tensor("lhsA", [48, 76], F32, kind="ExternalInput")
    lhsB_d = nc.dram_tensor("lhsB", [48, 44], F32, kind="ExternalInput")
    h0_d = nc.dram_tensor("h0", [12, BS], F32, kind="ExternalInput")
    ones_d = nc.dram_tensor("ones", [1, COLS], F32, kind="ExternalInput")
    out_d = nc.dram_tensor("out", [3, NTOK], F32, kind="ExternalOutput")
    hfin_d = nc.dram_tensor("hfin", [12, 32], F32, kind="ExternalOutput")

    with tile.TileContext(nc) as tc, ExitStack() as ctx:
        singles = ctx.enter_context(tc.tile_pool(name="singles", bufs=1))
        xpool = ctx.enter_context(tc.tile_pool(name="xp", bufs=2))
        p1psum = ctx.enter_context(tc.tile_pool(name="p1ps", bufs=2, space="PSUM"))
        psA = ctx.enter_context(tc.tile_pool(name="psA", bufs=2, space="PSUM"))
        psB = ctx.enter_context(tc.tile_pool(name="psB", bufs=2, space="PSUM"))
        psN = ctx.enter_context(tc.tile_pool(name="psN", bufs=2, space="PSUM"))
        work = ctx.enter_context(tc.tile_pool(name="wk", bufs=3))

        R = singles.tile([48, COLS], F32)
        lhsA = singles.tile([48, 76], F32)
        lhsB = singles.tile([48, 44], F32)
        w0 = singles.tile([128, 36], F32)
        nc.vector.memset(R[:, :], 0.0)
        nc.sync.dma_start(out=lhsA[:], in_=lhsA_d[:])
        nc.sync.dma_start(out=lhsB[:], in_=lhsB_d[:])
        nc.sync.dma_start(out=w0[:], in_=w0T_d[:])
        nc.sync.dma_start(out=R[41:42, :], in_=ones_d[:])
        # initial hidden state, staggered for the wavefront warm-up
        for c in range(4):
            nc.sync.dma_start(out=R[3 * c:12, 8 * c:8 * c + 8], in_=h0_d[3 * c:12, :])

        # ---- Phase 1: gx0 into R[32:41, 0:NTOK] ----
        # Interleaved with the scan below: the scan consumes gx0 column s at
        # step s, so each 512-token sub-block is emitted just before the
        # scan steps that need it, and each x block's DMAs are emitted one
        # block (256 steps) ahead. Engine idle time within the scan's
        # latency-bound steps absorbs the phase-1 work.
        XBLK = 2048
        n_blk = NTOK // XBLK
        xtiles = {}

        def emit_x_dmas(blk):
            tok0 = blk * XBLK
            xts = []
            for c in range(4):
                xt = xpool.tile([128, XBLK], F32, tag=f"x{c}")
                nc.sync.dma_start(
                    out=xt[:], in_=xT_d[128 * c:128 * c + 128, tok0:tok0 + XBLK]
                )
                xts.append(xt)
            xtiles[blk] = xts

        def emit_sub_block(sb):
            blk, sub = divmod(sb, XBLK // 512)
            tok0 = blk * XBLK
            xts = xtiles[blk]
            ps1 = p1psum.tile([9, 512], F32)
            for c in range(4):
                nc.tensor.matmul(
                    ps1[:],
                    w0[:, 9 * c:9 * c + 9],
                    xts[c][:, 512 * sub:512 * sub + 512],
                    start=(c == 0),
                    stop=(c == 3),
                )
            nc.scalar.copy(
                R[32:41, tok0 + 512 * sub:tok0 + 512 * sub + 512], ps1[:]
            )

        emit_x_dmas(0)
        emit_sub_block(0)
        n_sub = NTOK // 512

        # ---- Phase 2: wavefront scan ----
        # Steps 0..3 read h from R columns (warm-up: h0 rows pre-filled).
        # From step 4 on, the gate matmuls accumulate lhsT.T @ (a + u)
        # directly in PSUM (h_s = a_s + u_s), which drops the blend-add
        # from the loop-carried dependency cycle; the blend still runs
        # off-chain to materialize h in R for the z*h term and the output.
        prev_a = prev_un = None
        for s in range(scan_steps):
            if s > 0 and s % 64 == 0 and s // 64 < n_sub:
                emit_sub_block(s // 64)
            if s % 256 == 0 and s // 256 + 1 < n_blk:
                emit_x_dmas(s // 256 + 1)
            c0 = 8 * s
            pa = psA.tile([76, 8], F32)
            pb = psB.tile([44, 8], F32)
            if s < 4:
                nc.tensor.matmul(pa[:], lhsA[:], R[0:48, c0:c0 + 8],
                                 start=True, stop=True)
                nc.tensor.matmul(pb[:], lhsB[:], R[0:48, c0:c0 + 8],
                                 start=True, stop=True)
            else:
                nc.tensor.matmul(pa[:], lhsA[32:48, :], R[32:48, c0:c0 + 8],
                                 start=True, stop=False, skip_group_check=True)
                nc.tensor.matmul(pb[:], lhsB[32:48, :], R[32:48, c0:c0 + 8],
                                 start=True, stop=False, skip_group_check=True)
                nc.tensor.matmul(pa[:], lhsA[0:12, :], prev_un[:],
                                 start=False, stop=False, skip_group_check=True)
                nc.tensor.matmul(pb[:], lhsB[0:12, :], prev_un[:],
                                 start=False, stop=False, skip_group_check=True)
                nc.tensor.matmul(pa[:], lhsA[0:12, :], prev_a[:],
                                 start=False, stop=True, skip_group_check=True)
                nc.tensor.matmul(pb[:], lhsB[0:12, :], prev_a[:],
                                 start=False, stop=True, skip_group_check=True)
            rt = work.tile([12, 8], F32, tag="rt")
            nc.scalar.activation(rt[:], pa[0:12, :], AF.Sigmoid)
            zw = work.tile([44, 8], F32, tag="zw")
            nc.scalar.activation(zw[:], pb[0:44, :], AF.Sigmoid)
            tt = work.tile([12, 8], F32, tag="tt")
            nc.vector.tensor_mul(tt[:], rt[:], pa[64:76, :])
            un = work.tile([12, 8], F32, tag="un")
            nc.vector.tensor_mul(un[:], zw[0:12, :], R[0:12, c0:c0 + 8])
            sc = work.tile([12, 8], F32, tag="sc")
            nc.vector.tensor_add(sc[:], tt[:], pa[32:44, :])
            n = psN.tile([12, 8], F32)
            nc.scalar.activation(n[:], sc[:], AF.Tanh)
            a = work.tile([12, 8], F32, tag="a")
            nc.vector.tensor_mul(a[:], zw[32:44, :], n[:])
            rows = min(12, 3 * (s + 1))
            nc.vector.tensor_add(
                R[0:rows, c0 + 8:c0 + 16], a[0:rows, :], un[0:rows, :]
            )
            prev_a, prev_un = a, un

        nc.sync.dma_start(out=out_d[:], in_=R[9:12, 32:32 + NTOK])
        nc.sync.dma_start(out=hfin_d[:], in_=R[0:12, NTOK:NTOK + 32])

    nc.finalize()
    _BUILD_CACHE[key] = nc
    return nc


def build_lhs_ab(w_hh0, b_ih0, b_hh0, w_ih_rest, w_hh_rest, b_ih_rest, b_hh_rest):
    """Pack the scan's two stationary matrices.

    lhsA (48,76): cols 0:12 r, 32:44 nx, 64:76 nh.
    lhsB (48,44): cols 0:12 z, 32:44 w = 1-z (negated preactivation).
    Contract rows: 0:12 h, 32:41 gx0 [r,z,n], 41 ones.
    """
    A = np.zeros((48, 76), np.float64)
    Bm = np.zeros((48, 44), np.float64)
    for l in range(L):
        whh = w_hh0 if l == 0 else w_hh_rest[l - 1]
        bih = b_ih0 if l == 0 else b_ih_rest[l - 1]
        bhh = b_hh0 if l == 0 else b_hh_rest[l - 1]
        wih = None if l == 0 else w_ih_rest[l - 1]
        for g in range(3):
            c = 3 * l + g
            for j in range(3):
                A[3 * l + j, c] += whh[g, j]            # r
                A[3 * l + j, 64 + c] += whh[6 + g, j]   # nh
                Bm[3 * l + j, c] += whh[3 + g, j]       # z
                Bm[3 * l + j, 32 + c] -= whh[3 + g, j]  # w
                if l > 0:
                    A[3 * (l - 1) + j, c] += wih[g, j]
                    A[3 * (l - 1) + j, 32 + c] += wih[6 + g, j]
                    Bm[3 * (l - 1) + j, c] += wih[3 + g, j]
                    Bm[3 * (l - 1) + j, 32 + c] -= wih[3 + g, j]
            if l == 0:
                A[32 + g, c] = 1.0       # gx0 r -> r0
                A[38 + g, 32 + c] = 1.0  # gx0 n -> nx0
                Bm[35 + g, c] = 1.0      # gx0 z -> z0
                Bm[35 + g, 32 + c] = -1.0
            A[41, c] = bih[g] + bhh[g]
            A[41, 32 + c] = bih[6 + g]
            A[41, 64 + c] = bhh[6 + g]
            Bm[41, c] = bih[3 + g] + bhh[3 + g]
            Bm[41, 32 + c] = -(bih[3 + g] + bhh[3 + g])
    return (np.ascontiguousarray(A.astype(np.float32)),
            np.ascontiguousarray(Bm.astype(np.float32)))


def kernel(x, hxs, w_ih0, w_hh0, b_ih0, b_hh0, w_ih_rest, w_hh_rest,
           b_ih_rest, b_hh_rest):
    x = np.asarray(x, np.float32)
    hxs = np.asarray(hxs, np.float32)
    lhsA, lhsB = build_lhs_ab(
        np.asarray(w_hh0, np.float64), np.asarray(b_ih0, np.float64),
        np.asarray(b_hh0, np.float64), np.asarray(w_ih_rest, np.float64),
        np.asarray(w_hh_rest, np.float64), np.asarray(b_ih_rest, np.float64),
        np.asarray(b_hh_rest, np.float64),
    )
    w0T = np.ascontiguousarray(
        np.asarray(w_ih0, np.float32).T.reshape(4, 128, 9)
        .transpose(1, 0, 2).reshape(128, 36)
    )
    ones = np.ones((1, COLS), np.float32)
    in_maps = []
    for k in range(NC_CORES):
        sl = slice(BS * k, BS * k + BS)
        xT = np.ascontiguousarray(
            x[:, sl, :].transpose(2, 0, 1).reshape(IN, NTOK)
        )
        h0 = np.ascontiguousarray(
            hxs[:, sl, :].transpose(0, 2, 1).reshape(12, BS)
        )
        in_maps.append({"xT": xT, "w0T": w0T, "lhsA": lhsA, "lhsB": lhsB,
                        "h0": h0, "ones": ones})

    global _LAST_IN_MAPS
    _LAST_IN_MAPS = in_maps
    nc = _build_bass()
    results = run_bass_kernel_spmd(nc, in_maps, list(range(NC_CORES))).results

    outs, hs = [], []
    for k in range(NC_CORES):
        o = results[k]["out"].reshape(3, T, BS).transpose(1, 2, 0)
        outs.append(o)
        hf = results[k]["hfin"]
        hs.append(np.stack(
            [hf[3 * l:3 * l + 3, 8 * l:8 * l + 8].T for l in range(L)]
        ))
    out_full = np.ascontiguousarray(np.concatenate(outs, axis=1), dtype=np.float32)
    h_full = np.ascontiguousarray(np.concatenate(hs, axis=1), dtype=np.float32)
    return out_full, h_full
